# revision 4
# baseline (speedup 1.0000x reference)
"""BaseLSSFPN voxel-pooling (LSS lift-splat scatter-add) on 8 Trainium2 cores.

Strategy (data-parallel over B*N_cams, 1.5 cameras per core):
 - Host: per core, transpose its depth/context slices to (hw, .) layout and
   precompute an LSS-style scatter plan from geom_xyz (voxel index per point,
   counting-sorted by voxel into 128 voxel-blocks with padded slots). Index
   math on host mirrors real LSS deployments where frustum geometry is
   precomputed; all FP math runs on device.
 - Device (one NEFF, SPMD on 8 cores):
   Phase 1: softmax over depth bins; build a packed table in DRAM with one
     512B row per (hw position, depth-group of 14): [context(80) | depth(14)].
   Phase 2: dma_gather rows in sorted-by-voxel order; per 1024-slot gather
     call, batched DVE ops select each point\'s depth weight and build
     depth-weighted one-hots over the 128 x-positions; matmul-accumulate
     each 128-voxel block in PSUM; copy finished blocks into an SBUF BEV
     image [x=128, y*80+c].
 - Host: transpose per-core partial BEVs and sum the 4 cores of each batch.
"""

import math

import numpy as np

import concourse.bass as bass
import concourse.bacc as bacc
import concourse.mybir as mybir
from concourse.library_config import mlp
from concourse.tile import TileContext
from concourse.bass_utils import run_bass_kernel_spmd

# problem geometry
VX = VY = VZ = 128
B, NCAMS, D, H, W, C = 2, 6, 112, 16, 44, 80
NCORES = 8
HALF = H // 2          # 8 h-rows per half-frame
HWH = HALF * W         # 352 hw positions per half-frame
NHF = 3                # half-frames per core
HTOT = NHF * HWH       # 1056
HPAD = 1152            # 9 tiles of 128
NG, DGS = 8, 14        # 112 depth bins = 8 groups of 14
PROWS = HPAD * NG      # packed table rows
ELEM = 128             # padded row: 128 f32 = 512B
CTX_OFF, DEP_OFF = 0, 80
CHUNK_TILES = 8        # tiles per dma_gather call (1024 rows; ring limit ~1.5k)


def _plan_core(k, depth_logits, context, geom_xyz):
    depth_t = np.zeros((HPAD, D), np.float32)
    ctx_t = np.zeros((HPAD, C), np.float32)
    v_all = np.zeros((HTOT, D), np.int64)
    valid_all = np.zeros((HTOT, D), bool)
    batch = None
    for i in range(NHF):
        hf = NHF * k + i
        f, half = hf // 2, hf % 2
        b, cam = f // NCAMS, f % NCAMS
        batch = b if batch is None else batch
        assert batch == b
        sl = slice(half * HALF, (half + 1) * HALF)
        depth_t[i * HWH:(i + 1) * HWH] = (
            depth_logits[f][:, sl, :].reshape(D, HWH).T
        )
        ctx_t[i * HWH:(i + 1) * HWH] = context[f][:, sl, :].reshape(C, HWH).T
        g = geom_xyz[b, cam, :, sl, :, :]  # (D, HALF, W, 3)
        gx = g[..., 0].reshape(D, HWH).T.astype(np.int64)
        gy = g[..., 1].reshape(D, HWH).T.astype(np.int64)
        gz = g[..., 2].reshape(D, HWH).T.astype(np.int64)
        ok = (
            (gx >= 0) & (gx < VX) & (gy >= 0) & (gy < VY) & (gz >= 0) & (gz < VZ)
        )
        v_all[i * HWH:(i + 1) * HWH] = gy * VX + gx
        valid_all[i * HWH:(i + 1) * HWH] = ok

    h_arr, d_arr = np.nonzero(valid_all)
    vs = v_all[valid_all]
    order = np.argsort(vs, kind="stable")
    vs, hs, ds = vs[order], h_arr[order], d_arr[order]
    blocks = vs >> 7
    counts = np.bincount(blocks, minlength=VY)
    return dict(
        depth_t=depth_t, ctx_t=ctx_t, vs=vs, hs=hs, ds=ds, blocks=blocks,
        counts=counts, mt=math.ceil(counts.max() / 128), batch=batch,
    )


def _fill_streams(plan, m_tiles):
    slots_per_block = m_tiles * 128
    slots = VY * slots_per_block
    vs, hs, ds, blocks, counts = (
        plan["vs"], plan["hs"], plan["ds"], plan["blocks"], plan["counts"],
    )
    starts = np.zeros(VY, np.int64)
    starts[1:] = np.cumsum(counts)[:-1]
    rank = np.arange(len(vs)) - starts[blocks]
    slot = blocks * slots_per_block + rank

    gidx = np.zeros(slots, np.int16)
    gidx[slot] = (hs * NG + ds // DGS).astype(np.int16)
    drel = np.full(slots, -1.0, np.float32)
    drel[slot] = (ds % DGS).astype(np.float32)
    vrel = np.full(slots, -1000.0, np.float32)
    vrel[slot] = (vs & (VX - 1)).astype(np.float32)

    gidx_sb = np.ascontiguousarray(gidx.reshape(-1, 16).T)  # [16, slots//16]
    vrel_sb = np.ascontiguousarray(vrel.reshape(-1, 128).T).astype(np.int16)
    drel_sb = np.ascontiguousarray(drel.reshape(-1, 128).T).astype(np.int8)
    return dict(
        depth_t=plan["depth_t"], ctx_t=plan["ctx_t"],
        gidx=gidx_sb, vrel=vrel_sb, drel=drel_sb,
    )


def _build_nc(m_tiles, repeat=1, noop=False):
    slots = VY * m_tiles * 128
    n_tiles = slots // 128
    n_calls = n_tiles // CHUNK_TILES
    f32, i16 = mybir.dt.float32, mybir.dt.int16

    nc = bacc.Bacc(
        "TRN2", target_bir_lowering=False, debug=False, num_devices=NCORES,
        num_swdge_queues=4,
    )
    depth_h = nc.dram_tensor("depth_t", [HPAD, D], f32, kind="ExternalInput")
    ctx_h = nc.dram_tensor("ctx_t", [HPAD, C], f32, kind="ExternalInput")
    gidx_h = nc.dram_tensor("gidx", [16, slots // 16], i16, kind="ExternalInput")
    vrel_h = nc.dram_tensor("vrel", [128, n_tiles], i16, kind="ExternalInput")
    drel_h = nc.dram_tensor("drel", [128, n_tiles], mybir.dt.int8, kind="ExternalInput")
    bev_h = nc.dram_tensor("bev", [32, VY * C], f32, kind="ExternalOutput")
    packed = nc.dram_tensor("packed", [PROWS, ELEM], f32, kind="Internal")

    with TileContext(nc) as tc:
        with (
            tc.tile_pool(name="dram", bufs=1, space="DRAM") as dpool,
            tc.tile_pool(name="consts", bufs=1) as cpool,
            tc.tile_pool(name="p1", bufs=2) as p1,
            tc.tile_pool(name="gath", bufs=4) as gpool,
            tc.tile_pool(name="mrg", bufs=4) as mpool,
            tc.tile_pool(name="psum", bufs=8, space="PSUM") as psum_pool,
        ):
            nc.gpsimd.load_library(mlp)

            # resident streams / constants / output image
            gidx_t = cpool.tile([128, slots // 16], i16)
            vrel_i = cpool.tile([128, n_tiles], i16)
            drel_i = cpool.tile([128, n_tiles], mybir.dt.int8)
            vrel_t = cpool.tile([128, n_tiles], f32)
            drel_t = cpool.tile([128, n_tiles], f32)
            iota_i = cpool.tile([128, 128], mybir.dt.int32)
            iota_t = cpool.tile([128, 128], f32)
            bev_sb = cpool.tile([128, VY * C], f32)
            for g in range(8):
                nc.sync.dma_start(out=gidx_t[g * 16:(g + 1) * 16, :], in_=gidx_h[:])
            nc.sync.dma_start(out=vrel_i[:], in_=vrel_h[:])
            nc.sync.dma_start(out=drel_i[:], in_=drel_h[:])
            nc.vector.tensor_copy(out=vrel_t[:], in_=vrel_i[:])
            nc.vector.tensor_copy(out=drel_t[:], in_=drel_i[:])
            nc.gpsimd.iota(iota_i[:], pattern=[[1, 128]], base=0, channel_multiplier=0)
            nc.vector.tensor_copy(out=iota_t[:], in_=iota_i[:])

            reps = 0 if noop else repeat
            if noop:
                nc.vector.memset(bev_sb[:], 0.0)
            for _rep in range(reps):
                _phases(
                    nc, p1, gpool, mpool, psum_pool, m_tiles, n_calls,
                    depth_h, ctx_h, packed, gidx_t, vrel_t, drel_t, iota_t, bev_sb,
                )
            cc_in = dpool.tile([128, VY * C], f32)
            cc_out = dpool.tile([32, VY * C], f32)
            nc.gpsimd.dma_start(out=cc_in[:], in_=bev_sb[:])
            nc.gpsimd.collective_compute(
                "ReduceScatter", mybir.AluOpType.add,
                replica_groups=[[0, 1, 2, 3], [4, 5, 6, 7]],
                ins=[cc_in.opt()], outs=[cc_out.opt()],
            )
            nc.gpsimd.dma_start(out=bev_h[:], in_=cc_out[:])

    nc.compile()
    return nc


def _phases(
    nc, p1, gpool, mpool, psum_pool, m_tiles, n_calls,
    depth_h, ctx_h, packed, gidx_t, vrel_t, drel_t, iota_t, bev_sb,
):
    f32 = mybir.dt.float32
    # ---- Phase 1: softmax + packed table ----
    for ht in range(HPAD // 128):
        dep = p1.tile([128, D], f32, tag="dep")
        ctx2 = p1.tile([128, C], f32, tag="ctx")
        nc.sync.dma_start(out=dep[:], in_=depth_h[ht * 128:(ht + 1) * 128, :])
        nc.sync.dma_start(out=ctx2[:], in_=ctx_h[ht * 128:(ht + 1) * 128, :])
        negmax = p1.tile([128, 1], f32, tag="negmax")
        nc.vector.reduce_max(
            out=negmax[:], in_=dep[:], axis=mybir.AxisListType.X, negate=True,
        )
        expd = p1.tile([128, D], f32, tag="expd")
        sumd = p1.tile([128, 1], f32, tag="sumd")
        nc.scalar.activation(
            out=expd[:], in_=dep[:], func=mybir.ActivationFunctionType.Exp,
            bias=negmax[:, 0:1], scale=1.0, accum_out=sumd[:],
        )
        pk = p1.tile([128, NG, ELEM], f32, tag="pk")
        pk3 = pk[:]
        # context replicated into each depth-group row
        nc.vector.tensor_copy(
            out=pk3[:, :, CTX_OFF:CTX_OFF + C],
            in_=ctx2[:].rearrange("p (o c) -> p o c", o=1).broadcast_to(
                [128, NG, C]
            ),
        )
        # normalized depth split into groups of 14
        rec = p1.tile([128, 1], f32, tag="rec")
        nc.vector.reciprocal(out=rec[:], in_=sumd[:])
        nc.vector.tensor_scalar(
            out=pk3[:, :, DEP_OFF:DEP_OFF + DGS],
            in0=expd[:].rearrange("p (g r) -> p g r", g=NG),
            scalar1=rec[:, 0:1], scalar2=None,
            op0=mybir.AluOpType.mult,
        )
        nc.sync.dma_start(
            out=packed[ht * 128 * NG:(ht + 1) * 128 * NG, :].rearrange(
                "(p g) e -> p (g e)", p=128
            ),
            in_=pk[:].rearrange("p g e -> p (g e)"),
        )

    # ---- Phase 2: gather + merge ----
    CT = CHUNK_TILES
    for call in range(n_calls):
        t0 = call * CT
        gt = gpool.tile([128, CT, ELEM], f32, tag="gt")
        nc.gpsimd.dma_gather(
            gt[:], packed[:],
            gidx_t[:, t0 * 8:(t0 + CT) * 8],
            CT * 128, CT * 128, ELEM,
            queue_num=call % 4,
        )
        # batched depth select: dsel8[p, t] = deprow[p, t, drel[p, t]]
        wm = mpool.tile([128, CT, DGS], f32, tag="wm")
        nc.vector.tensor_tensor(
            out=wm[:],
            in0=iota_t[:, :DGS].rearrange("p (o r) -> p o r", o=1).broadcast_to(
                [128, CT, DGS]
            ),
            in1=drel_t[:, t0:t0 + CT].rearrange("p (t o) -> p t o", o=1).broadcast_to(
                [128, CT, DGS]
            ),
            op=mybir.AluOpType.is_equal,
        )
        nc.vector.tensor_tensor(
            out=wm[:], in0=wm[:], in1=gt[:, :, DEP_OFF:DEP_OFF + DGS],
            op=mybir.AluOpType.mult,
        )
        dsel8 = mpool.tile([128, CT], f32, tag="dsel8")
        nc.vector.reduce_sum(out=dsel8[:], in_=wm[:], axis=mybir.AxisListType.X)
        # batched one-hot M: m8[p, t, q] = (iota[q] == vrel[p,t]) * dsel8[p,t]
        m8 = mpool.tile([128, CT, 128], f32, tag="m8")
        nc.vector.tensor_tensor(
            out=m8[:],
            in0=iota_t[:].rearrange("p (o q) -> p o q", o=1).broadcast_to(
                [128, CT, 128]
            ),
            in1=vrel_t[:, t0:t0 + CT].rearrange("p (t o) -> p t o", o=1).broadcast_to(
                [128, CT, 128]
            ),
            op=mybir.AluOpType.is_equal,
        )
        nc.vector.tensor_tensor(
            out=m8[:], in0=m8[:],
            in1=dsel8[:].rearrange("p (t o) -> p t o", o=1).broadcast_to(
                [128, CT, 128]
            ),
            op=mybir.AluOpType.mult,
        )
        for j in range(CT):
            t = t0 + j
            blk, jj = t // m_tiles, t % m_tiles
            if jj == 0:
                ps = psum_pool.tile([128, C], f32, tag="blk")
            nc.tensor.matmul(
                out=ps[:], lhsT=m8[:, j, :], rhs=gt[:, j, CTX_OFF:CTX_OFF + C],
                start=(jj == 0), stop=(jj == m_tiles - 1),
            )
            if jj == m_tiles - 1:
                nc.scalar.copy(out=bev_sb[:, blk * C:(blk + 1) * C], in_=ps[:])


_NC_CACHE = {}
LAST_RESULTS = None  # set by kernel(); test harness reads exec_time_ns/profile


def kernel(depth_logits, context, geom_xyz):
    global LAST_RESULTS
    depth_logits = np.asarray(depth_logits, np.float32)
    context = np.asarray(context, np.float32)
    geom_xyz = np.asarray(geom_xyz, np.int32)

    plans = [_plan_core(k, depth_logits, context, geom_xyz) for k in range(NCORES)]
    m_tiles = max(8, max(p["mt"] for p in plans))
    if m_tiles not in _NC_CACHE:
        _NC_CACHE[m_tiles] = _build_nc(m_tiles)
    nc = _NC_CACHE[m_tiles]

    in_maps = [_fill_streams(p, m_tiles) for p in plans]
    res = run_bass_kernel_spmd(nc, in_maps, core_ids=list(range(NCORES)))
    LAST_RESULTS = res

    out = np.zeros((B, C, VY, VX), np.float32)
    for k in range(NCORES):
        part = res.results[k]["bev"].reshape(32, VY, C)  # [x_local, y, c]
        x0 = 32 * (k % 4)
        out[plans[k]["batch"], :, :, x0:x0 + 32] = part.transpose(2, 1, 0)
    return out


def bench(inputs, reps=(1, 5), iters=6):
    """Min wall time per repeat-variant NEFF; slope vs reps = phase time."""
    import time

    depth_logits = np.asarray(inputs["depth_logits"], np.float32)
    context = np.asarray(inputs["context"], np.float32)
    geom_xyz = np.asarray(inputs["geom_xyz"], np.int32)
    plans = [_plan_core(k, depth_logits, context, geom_xyz) for k in range(NCORES)]
    m_tiles = max(8, max(p["mt"] for p in plans))
    in_maps = [_fill_streams(p, m_tiles) for p in plans]

    out = []
    for r in reps:
        key = (m_tiles, r)
        if key not in _NC_CACHE:
            _NC_CACHE[key] = _build_nc(m_tiles, repeat=r, noop=(r == 0))
        nc = _NC_CACHE[key]
        best = float("inf")
        run_bass_kernel_spmd(nc, in_maps, core_ids=list(range(NCORES)))  # warm
        for _ in range(iters):
            t0 = time.time()
            run_bass_kernel_spmd(nc, in_maps, core_ids=list(range(NCORES)))
            best = min(best, time.time() - t0)
        out.append(best)
    return out



# revision 10
# speedup vs baseline: 3.2982x; 3.2982x over previous
"""BaseLSSFPN voxel pooling v3: gather-by-voxel-slot + batched DVE reduce.

Measured laws of this axon/trn2 runtime: dependency-free same-engine
instruction streams are nearly free; cross-engine syncs / DMA-completion
waits cost ~0.2-1 ms; dma_gather calls pipeline cheaply. So:
 - Phase 1 (8 instructions): softmax + packed bf16 table [9217, 128]
   ([ctx 80 | dep-group 14 | pad], one row per (hw, depth-group), +zero row).
 - Host assigns every point to slot (v-section, occurrence r, y, x) with
   x = v%128 on the gather's partition axis; empty slots gather the zero row.
 - Per v-section: ~40 dma_gather calls fill buf [128, 16*R, 128]; 5 big DVE
   ops select the depth weight and scale ctx; 1 strided reduce over r
   accumulates into the BEV image [x, y, c] - the segment-sum with no
   scatter primitive and no races.
 - Same ReduceScatter + host unshard as the baseline.
"""

import math

import numpy as np

import concourse.bass as bass
import concourse.bacc as bacc
import concourse.mybir as mybir
from concourse.library_config import mlp
from concourse.tile import TileContext
from concourse.bass_utils import run_bass_kernel_spmd

VX = VY = VZ = 128
B, NCAMS, D, H, W, C = 2, 6, 112, 16, 44, 80
NCORES = 8
HALF = H // 2
HWH = HALF * W
NHF = 3
HTOT = NHF * HWH        # 1056
HPAD = 1152
NT = HPAD // 128        # 9
NG, DGS = 8, 14         # 112 = 8 groups of 14
ROWS = HPAD * NG        # 9216 table rows
ZROW = ROWS             # zero row id
NV = VX * VY
NSEC = 8                # v-sections of 2048 voxels (16 y-rows each)
SECV = NV // NSEC
CT = 8                  # gather tiles per call


def _plan3(k, depth_logits, context, geom_xyz):
    depth_t = np.zeros((HPAD, D), np.float32)
    ctx_t = np.zeros((HPAD, C), np.float32)
    v_all = np.zeros((HTOT, D), np.int64)
    valid_all = np.zeros((HTOT, D), bool)
    batch = None
    for i in range(NHF):
        hf = NHF * k + i
        f, half = hf // 2, hf % 2
        b, cam = f // NCAMS, f % NCAMS
        batch = b if batch is None else batch
        sl = slice(half * HALF, (half + 1) * HALF)
        depth_t[i * HWH:(i + 1) * HWH] = (
            depth_logits[f][:, sl, :].reshape(D, HWH).T
        )
        ctx_t[i * HWH:(i + 1) * HWH] = context[f][:, sl, :].reshape(C, HWH).T
        g = geom_xyz[b, cam, :, sl, :, :]
        gx = g[..., 0].reshape(D, HWH).T.astype(np.int64)
        gy = g[..., 1].reshape(D, HWH).T.astype(np.int64)
        gz = g[..., 2].reshape(D, HWH).T.astype(np.int64)
        ok = (
            (gx >= 0) & (gx < VX) & (gy >= 0) & (gy < VY) & (gz >= 0) & (gz < VZ)
        )
        v_all[i * HWH:(i + 1) * HWH] = gy * VX + gx
        valid_all[i * HWH:(i + 1) * HWH] = ok

    h_arr, d_arr = np.nonzero(valid_all)
    vs = v_all[valid_all]
    order = np.argsort(vs, kind="stable")
    vs, hs, ds = vs[order], h_arr[order], d_arr[order]
    counts = np.bincount(vs, minlength=NV)          # points per voxel
    starts = np.zeros(NV + 1, np.int64)
    starts[1:] = np.cumsum(counts)
    rank = np.arange(len(vs)) - starts[vs]
    # per-section max occupancy
    r_sec = counts.reshape(NSEC, SECV).max(axis=1)
    return dict(
        depth_t=depth_t, ctx_t=ctx_t, vs=vs, hs=hs, ds=ds, rank=rank,
        r_sec=r_sec, batch=batch,
    )


def _fill3(plan, r_list):
    tiles_sec = [16 * r for r in r_list]
    tile_off = np.concatenate([[0], np.cumsum(tiles_sec)])
    n_tiles = int(tile_off[-1])
    slots = n_tiles * 128
    vs, hs, ds, rank = plan["vs"], plan["hs"], plan["ds"], plan["rank"]

    sec = vs // SECV
    y = (vs % SECV) // VX
    x = vs % VX
    tile = tile_off[sec] + 16 * rank + y
    slot = tile * 128 + x

    gidx = np.full(slots, ZROW, np.int16)
    gidx[slot] = (hs * NG + ds // DGS).astype(np.int16)
    drel = np.zeros(slots, np.int8)
    drel[slot] = (ds % DGS).astype(np.int8)

    gidx_sb = np.ascontiguousarray(gidx.reshape(-1, 16).T)      # [16, slots/16]
    drel_sb = np.ascontiguousarray(drel.reshape(-1, 128).T)     # [128, n_tiles]
    return dict(
        depth_t=plan["depth_t"], ctx_t=plan["ctx_t"],
        gidx=gidx_sb, drel=drel_sb,
    )


def _build3(r_list, repeat=1):
    f32, i16, i8 = mybir.dt.float32, mybir.dt.int16, mybir.dt.int8
    bf16 = mybir.dt.bfloat16
    tiles_sec = [16 * r for r in r_list]
    n_tiles = int(sum(tiles_sec))
    slots = n_tiles * 128

    nc = bacc.Bacc(
        "TRN2", target_bir_lowering=False, debug=False, num_devices=NCORES,
        num_swdge_queues=4,
    )
    depth_h = nc.dram_tensor("depth_t", [HPAD, D], f32, kind="ExternalInput")
    ctx_h = nc.dram_tensor("ctx_t", [HPAD, C], f32, kind="ExternalInput")
    gidx_h = nc.dram_tensor("gidx", [16, slots // 16], i16, kind="ExternalInput")
    drel_h = nc.dram_tensor("drel", [128, n_tiles], i8, kind="ExternalInput")
    bev_h = nc.dram_tensor("bev", [32, VY * C], f32, kind="ExternalOutput")
    packed = nc.dram_tensor("packed", [ROWS + 16, 128], bf16, kind="Internal")

    with TileContext(nc) as tc:
        with (
            tc.tile_pool(name="dram", bufs=1, space="DRAM") as dpool,
            tc.tile_pool(name="consts", bufs=1) as cpool,
            tc.tile_pool(name="sec", bufs=1) as spool,
        ):
            nc.gpsimd.load_library(mlp)

            drel_i = cpool.tile([128, n_tiles], i8)
            drel_t = cpool.tile([128, n_tiles], bf16)
            iota_i = cpool.tile([128, 16], mybir.dt.int32)
            iota_t = cpool.tile([128, 16], bf16)
            dep_in = cpool.tile([128, NT, D], f32)
            ctx_in = cpool.tile([128, NT, C], f32)
            pk = cpool.tile([128, NT, NG, 128], bf16)
            bev_acc = cpool.tile([128, VY, 128], bf16)
            zrow = cpool.tile([128, 128], bf16)

            nc.sync.dma_start(out=drel_i[:], in_=drel_h[:])
            nc.vector.tensor_copy(out=drel_t[:], in_=drel_i[:])
            nc.gpsimd.iota(iota_i[:], pattern=[[1, 16]], base=0, channel_multiplier=0)
            nc.vector.tensor_copy(out=iota_t[:], in_=iota_i[:])
            nc.sync.dma_start(
                out=dep_in[:], in_=depth_h[:].rearrange("(t p) d -> p t d", p=128)
            )
            nc.sync.dma_start(
                out=ctx_in[:], in_=ctx_h[:].rearrange("(t p) c -> p t c", p=128)
            )
            nc.vector.memset(zrow[:], 0.0)
            nc.sync.dma_start(out=packed[ROWS:ROWS + 16, :], in_=zrow[0:16, :])

            for _rep in range(repeat):
                # ---- phase 1: softmax + packed table (few giant ops) ----
                expd = spool.tile([128, NT, D], f32, tag="buf")
                sums = spool.tile([128, NT], f32, tag="sums")
                rec = spool.tile([128, NT], f32, tag="rec")
                nc.scalar.activation(
                    out=expd[:], in_=dep_in[:],
                    func=mybir.ActivationFunctionType.Exp,
                )
                nc.vector.reduce_sum(
                    out=sums[:], in_=expd[:], axis=mybir.AxisListType.X,
                )
                nc.vector.reciprocal(out=rec[:], in_=sums[:])
                nc.vector.tensor_copy(
                    out=pk[:, :, :, 0:C],
                    in_=ctx_in[:].rearrange("p t (o c) -> p t o c", o=1)
                    .broadcast_to([128, NT, NG, C]),
                )
                nc.vector.tensor_tensor(
                    out=pk[:, :, :, C:C + DGS],
                    in0=expd[:].rearrange("p t (g r) -> p t g r", g=NG),
                    in1=rec[:].rearrange("p (t o q) -> p t o q", o=1, q=1)
                    .broadcast_to([128, NT, NG, DGS]),
                    op=mybir.AluOpType.mult,
                )
                nc.vector.memset(pk[:, :, :, C + DGS:128], 0.0)
                nc.sync.dma_start(
                    out=packed[0:ROWS, :].rearrange(
                        "(t p g) e -> p t g e", p=128, g=NG
                    ),
                    in_=pk[:],
                )

                # ---- per-section gather + select + reduce ----
                t0 = 0
                for s in range(NSEC):
                    nts = tiles_sec[s]
                    buf = spool.tile([128, 16 * max(r_list), 128], bf16, tag="buf")
                    gsec = spool.tile([128, nts * 8], i16, tag="gsec")
                    for g in range(8):
                        nc.sync.dma_start(
                            out=gsec[g * 16:(g + 1) * 16, :],
                            in_=gidx_h[:, t0 * 8:(t0 + nts) * 8],
                        )
                    n_calls = math.ceil(nts / CT)
                    for cix in range(n_calls):
                        a = cix * CT
                        ct = min(CT, nts - a)
                        nc.gpsimd.dma_gather(
                            buf[:, a:a + ct, :], packed[:],
                            gsec[:, a * 8:(a + ct) * 8],
                            ct * 128, ct * 128, 128,
                            queue_num=cix % 4,
                        )
                    # depth select: wm = (iota14 == drel) * dep_cols
                    wm = spool.tile([128, 16 * max(r_list), DGS], bf16, tag="wm")
                    nc.vector.tensor_tensor(
                        out=wm[:, 0:nts, :],
                        in0=iota_t[:, 0:DGS]
                        .rearrange("p (o r) -> p o r", o=1)
                        .broadcast_to([128, nts, DGS]),
                        in1=drel_t[:, t0:t0 + nts]
                        .rearrange("p (t o) -> p t o", o=1)
                        .broadcast_to([128, nts, DGS]),
                        op=mybir.AluOpType.is_equal,
                    )
                    nc.vector.tensor_tensor(
                        out=wm[:, 0:nts, :], in0=wm[:, 0:nts, :],
                        in1=buf[:, 0:nts, C:C + DGS],
                        op=mybir.AluOpType.mult,
                    )
                    dsel = spool.tile([128, 16 * max(r_list)], f32, tag="dsel")
                    nc.vector.reduce_sum(
                        out=dsel[:, 0:nts], in_=wm[:, 0:nts, :],
                        axis=mybir.AxisListType.X,
                    )
                    nc.vector.tensor_tensor(
                        out=buf[:, 0:nts, 0:C], in0=buf[:, 0:nts, 0:C],
                        in1=dsel[:, 0:nts]
                        .rearrange("p (t o) -> p t o", o=1)
                        .broadcast_to([128, nts, C]),
                        op=mybir.AluOpType.mult,
                    )
                    # segment-sum: reduce over r of view [p, y, e, r]
                    with nc.allow_low_precision(
                        reason="bf16 BEV partial sums, tol 2e-2"
                    ):
                        nc.vector.reduce_sum(
                            out=bev_acc[:, s * 16:(s + 1) * 16, :],
                            in_=buf[:, 0:nts, :].rearrange(
                                "p (r y) e -> p y e r", y=16
                            ),
                            axis=mybir.AxisListType.X,
                        )
                    t0 += nts

            # compact [x, y, c] -> [x, y*80+c]
            cmp_t = spool.tile([128, VY * C], bf16, tag="buf")
            nc.vector.tensor_copy(out=cmp_t[:], in_=bev_acc[:, :, 0:C])
            cc_in = dpool.tile([128, VY * C], bf16)
            cc_out = dpool.tile([32, VY * C], bf16)
            nc.gpsimd.dma_start(out=cc_in[:], in_=cmp_t[:])
            nc.gpsimd.collective_compute(
                "ReduceScatter", mybir.AluOpType.add,
                replica_groups=[[0, 1, 2, 3], [4, 5, 6, 7]],
                ins=[cc_in.opt()], outs=[cc_out.opt()],
            )
            nc.gpsimd.dma_start(out=bev_h[:], in_=cc_out[:])

    nc.compile()
    return nc


_NC3 = {}
LAST_RESULTS = None


def _prep(depth_logits, context, geom_xyz):
    depth_logits = np.asarray(depth_logits, np.float32)
    context = np.asarray(context, np.float32)
    geom_xyz = np.asarray(geom_xyz, np.int32)
    plans = [_plan3(k, depth_logits, context, geom_xyz) for k in range(NCORES)]
    r_list = tuple(
        int(max(p["r_sec"][s] for p in plans)) for s in range(NSEC)
    )
    in_maps = [_fill3(p, r_list) for p in plans]
    return plans, r_list, in_maps


def kernel(depth_logits, context, geom_xyz):
    global LAST_RESULTS
    plans, r_list, in_maps = _prep(depth_logits, context, geom_xyz)
    if r_list not in _NC3:
        _NC3[r_list] = _build3(r_list, repeat=1)
    nc = _NC3[r_list]
    res = run_bass_kernel_spmd(nc, in_maps, core_ids=list(range(NCORES)))
    LAST_RESULTS = res

    out = np.zeros((B, C, VY, VX), np.float32)
    for k in range(NCORES):
        part = res.results[k]["bev"].astype(np.float32).reshape(32, VY, C)
        x0 = 32 * (k % 4)
        out[plans[k]["batch"], :, :, x0:x0 + 32] = part.transpose(2, 1, 0)
    return out


def bench(inputs, reps=(1, 5), iters=6, knobs=()):
    import time

    plans, r_list, in_maps = _prep(
        inputs["depth_logits"], inputs["context"], inputs["geom_xyz"]
    )
    out = []
    for r in reps:
        key = (r_list, r)
        if key not in _NC3:
            _NC3[key] = _build3(r_list, repeat=r)
        nc = _NC3[key]
        best = float("inf")
        run_bass_kernel_spmd(nc, in_maps, core_ids=list(range(NCORES)))
        for _ in range(iters):
            t0 = time.time()
            run_bass_kernel_spmd(nc, in_maps, core_ids=list(range(NCORES)))
            best = min(best, time.time() - t0)
        out.append(best)
    return out
